# revision 48
# baseline (speedup 1.0000x reference)
"""Fused Linear + GroupNorm + Hardtanh kernel for Trainium2 (8 NeuronCores).

Problem: out = clip(groupnorm(x @ W.T + b, 32 groups), -2, 2), with
x [65536, 512] fp32, W [1024, 512] fp32, gamma=1/beta=0.

Strategy (data-parallel over the 8 cores, 8192 rows each):
 - Host pre-transposes x, casts matmul operands to fp16 (PSUM accum
   stays fp32), and packs x.T into per-group tiles: one [128, 4*512]
   SBUF tile per 4 m-tiles, kt-major, so each group is ONE 128-descriptor
   DMA (~650ns queue dispatch) and every matmul/stats slice is a simple
   contiguous slice of it.  W.T is packed the same way ([128, 4096],
   h0 halves then h1 halves) and stays SBUF-resident.
 - Group sums of y come from a second, *transposed* stats matmul per
   4-tile group (stationary = per-k-tile group-summed weights scaled by
   -1/32, moving = the group's 512 x columns), so the [-mean | 1]
   stationary needed by the mean/bias injection comes out of PSUM
   already in [group, m] layout.  The constant ones rows are added by
   the PSUM->SBUF staging copy (activation Identity with a
   per-partition bias mask).  Bias enters via the injection's ones row
   as b'' = b - groupmean(b).
 - The injection (rank-17 matmul per N-half) lands (b - mean) into the
   y PSUM, so the epilogue is: square per half (Scalar, fp16 out) ->
   one segmented fp16 tensor_reduce (Vector) -> sqrt (Scalar) -> one
   fused one-Newton-reciprocal + scale + hardtanh-clip custom DVE op
   per half writing fp16 -> DMA (sync queue).
 - y PSUM is split into independent h0/h1 pools (3 x 1 bank each,
   + 2 stats banks = 8) with [h0 x4][stats][h1 x4] matmul emission.
   Per-engine emission runs oldest-pipeline-stage first.
 - Startup: a few zero warmup matmuls start the PE p-state ramp while
   the first DMAs land; queue order puts the first group + kt0 weights
   ahead of everything else.
 - Output is written fp16 and widened to fp32 on the host.
"""
import sys

sys.path.insert(0, "/opt/trn_rl_repo")

import numpy as np

M_FULL, K, N = 65536, 512, 1024
NG, GS = 32, 32
EPS = 1e-5
HT = 2.0
N_CORES = 8
KT = K // 128  # 4 k-tiles
GRP = 4  # m-tiles per group (one x DMA + one stats matmul per group)
SW = 49  # stats width: [16 groups | ones | pad...] @0, [16 groups | ones] @32
N_WARM = 6  # zero matmuls that start the PE p-state ramp during startup

_custom_ops = {}


def _register_custom_ops():
    """Add the fused scale+clip DVE op to the custom-op table (idempotent)."""
    if _custom_ops:
        return _custom_ops
    import concourse.dve_ops as dve_ops
    from concourse.dve_spec import Spec, Src0, Src1, C0, C1, C2, Zero, minn, \
        maxx, lower, _has_src1
    from concourse.dve_uop import DveOpSpec

    def register(name, spec):
        if name in dve_ops._SUB_OPCODE_FOR_NAME:
            return next(o for o in dve_ops.OPS if o.name == name)
        row = max(dve_ops._SUB_OPCODE_FOR_NAME.values()) + 1
        assert row < 0x20
        op = dve_ops.DveOp(name, spec, subdim=False, uops_sha={})
        dve_ops.OPS.append(op)
        dve_ops._SUB_OPCODE_FOR_NAME[name] = row
        dve_ops.CUSTOM_DVE_SPECS[name] = spec
        for ver in ("v3", "v4"):
            uops = lower(spec, ver=ver)
            op.uops_sha[ver] = DveOpSpec(
                name=name, opcode=row, uops=uops,
                rd1_en=_has_src1(spec)).sha(ver)
        return op

    # out = clip(in0 / in1, -imm2, imm2): one-Newton fast reciprocal of the
    # broadcast group-std (Src1) fused with the scale and the hardtanh clip.
    # 8/8 ALU stages; reciprocal rel err ~1.7e-3.
    from concourse.dve_spec import Bin, AluOp
    y0 = Bin(AluOp.BITWISE_NOT, Src1, Src1) * C0
    y1 = y0 * (C1 - Src1 * y0)
    # the clip reuses the Newton constant C1=2.0017324 as the bound (8-stage
    # budget): clipping at +-2.0017 instead of +-2.0 adds <=1.7e-3 abs error

    def _ref_apply(in0, in1, s0, s1, imm2):
        x = np.ascontiguousarray(in1.astype(np.float32))
        nx = (~x.view(np.int32)).view(np.float32)
        y0r = nx * s0
        y1r = y0r * (s1 - x * y0r)
        return np.minimum(np.maximum(in0.astype(np.float32) * y1r, -s1), s1)

    _custom_ops["apply"] = register("APPLY_RECIP_CLIP_ANT", Spec(
        body=minn(maxx(Src0 * y1, Zero - C1), C1),
        reference=_ref_apply))
    return _custom_ops


def build(m_loc: int, apply_affine: bool):
    import concourse.bass as bass
    import concourse.mybir as mybir
    import concourse.tile as tile
    from concourse import bacc
    from contextlib import ExitStack

    ops = _register_custom_ops()
    f32 = mybir.dt.float32
    f16 = mybir.dt.float16
    Alu = mybir.AluOpType
    n_tiles = m_loc // 128
    gsz = min(GRP, n_tiles)  # m-tiles per group
    gc = gsz * 128  # x columns per group
    n_groups = n_tiles // gsz

    nc = bacc.Bacc()
    # x.T packed per group: row g*128+p, col kt*gc+c = x[g*gc+c, kt*128+p]
    xtp_d = nc.dram_tensor("xtp", [n_groups * 128, KT * gc], f16,
                           kind="ExternalInput")
    # packed weights: [128, 4096] = kt-major h0 halves then h1 halves
    wtp_d = nc.dram_tensor("wtp", [128, 2 * KT * 512], f16,
                           kind="ExternalInput")
    wgb_d = nc.dram_tensor("wgb", [128, KT * SW], f16, kind="ExternalInput")
    gb_d = nc.dram_tensor("gb", [SW, N], f16, kind="ExternalInput")
    msk_d = nc.dram_tensor("msk", [128, 1], f32, kind="ExternalInput")
    if apply_affine:
        gam_d = nc.dram_tensor("gam", [128, N], f32, kind="ExternalInput")
        bet_d = nc.dram_tensor("bet", [128, N], f32, kind="ExternalInput")
    out_d = nc.dram_tensor("out", [m_loc, N], f16, kind="ExternalOutput")

    with tile.TileContext(nc) as tc, ExitStack() as ctx:
        const = ctx.enter_context(tc.tile_pool(name="const", bufs=1))
        xpool = ctx.enter_context(tc.tile_pool(name="xts", bufs=4))
        # pool declaration order sets PSUM bank placement: stats at bank 0,
        # h0 in banks 1-3, h1 in banks 4-7.  Measured: this keeps BOTH the
        # stats matmuls (216ns) and the injection pairs (~320ns) at full
        # rate; pph0=4 or stats-last configs slow one of them by ~50%.
        pps = ctx.enter_context(tc.tile_pool(name="pps", bufs=1, space="PSUM"))
        pph0 = ctx.enter_context(tc.tile_pool(name="pph0", bufs=3,
                                              space="PSUM"))
        pph1 = ctx.enter_context(tc.tile_pool(name="pph1", bufs=4,
                                              space="PSUM"))
        epi = ctx.enter_context(tc.tile_pool(name="epi", bufs=3))
        extp = ctx.enter_context(tc.tile_pool(name="extp", bufs=3))
        outp = ctx.enter_context(tc.tile_pool(name="outp", bufs=4))

        # --- PE p-state warmup: zero matmuls keep the Tensor engine busy
        # (ramping its clock) while the first weight/x DMAs land ---
        # gpsimd memset issues ~1.3us earlier than vector at boot, so the
        # PE clock ramp starts sooner
        warm_sb = const.tile([128, 512], f16, tag="warm")
        nc.gpsimd.memset(warm_sb[:], 0.0)
        warm_ps = pps.tile([SW, 512], f32, tag="pt")
        for _ in range(N_WARM):
            nc.tensor.matmul(warm_ps[0:1, :], warm_sb[:, 0:1], warm_sb[:],
                             start=True, stop=True)

        # --- resident constants + first x groups.  Queue order = startup
        # priority: scalar carries group 0 then the small stats/inject
        # constants; sync carries the weights then group 1. ---
        xg = [None] * n_groups
        # group 0 arrives in kt-sized pieces (disjoint column ranges) so the
        # first main matmuls gate on as little data as possible
        xg[0] = xpool.tile([128, KT * gc], f16, tag="xts", name="xg0")
        nc.scalar.dma_start(out=xg[0][:, 0:gc], in_=xtp_d[0:128, 0:gc])
        nc.scalar.dma_start(out=xg[0][:, gc:2 * gc],
                            in_=xtp_d[0:128, gc:2 * gc])
        nc.scalar.dma_start(out=xg[0][:, 2 * gc:4 * gc],
                            in_=xtp_d[0:128, 2 * gc:4 * gc])
        wgb_sb = const.tile([128, KT * SW], f16, tag="wgb")
        nc.scalar.dma_start(out=wgb_sb[:], in_=wgb_d[:])
        # per-partition bias mask for the staging copy: 1.0 at the ones rows
        ones_sb = const.tile([128, 1], f32, tag="onesmask")
        nc.scalar.dma_start(out=ones_sb[:], in_=msk_d[:])
        gb_sb = const.tile([SW, N], f16, tag="gb")
        nc.scalar.dma_start(out=gb_sb[:], in_=gb_d[:])

        wt_sb = const.tile([128, 2 * KT * 512], f16, tag="wt")
        nc.sync.dma_start(out=wt_sb[:, 0:512], in_=wtp_d[:, 0:512])
        nc.sync.dma_start(out=wt_sb[:, 512:2048], in_=wtp_d[:, 512:2048])
        nc.sync.dma_start(out=wt_sb[:, 2048:2560], in_=wtp_d[:, 2048:2560])
        nc.sync.dma_start(out=wt_sb[:, 2560:4096], in_=wtp_d[:, 2560:4096])
        if n_groups > 1:
            xg[1] = xpool.tile([128, KT * gc], f16, tag="xts", name="xg1")
            nc.sync.dma_start(out=xg[1][:], in_=xtp_d[128:256, :])

        eps_sb = const.tile([128, 1], f32, tag="eps")
        nc.vector.memset(eps_sb[:], EPS)
        if apply_affine:
            gam_sb = const.tile([128, N], f32, tag="gam")
            nc.scalar.dma_start(out=gam_sb[:], in_=gam_d[:])
            bet_sb = const.tile([128, N], f32, tag="bet")
            nc.scalar.dma_start(out=bet_sb[:], in_=bet_d[:])

        state_a = {}
        state_b = {}
        cur = {"pt": None}

        def wslice(kt, h):
            return wt_sb[:, (h * KT + kt) * 512:(h * KT + kt + 1) * 512]

        def emit_main(mt):
            g, gloc = divmod(mt, gsz)
            if gloc == 0 and g + 2 < n_groups and xg[g + 2] is None:
                # prefetch group g+2 (4-deep pool: target buffer long freed)
                t = xpool.tile([128, KT * gc], f16, tag="xts", name="xgp")
                nc.sync.dma_start(
                    out=t[:], in_=xtp_d[(g + 2) * 128:(g + 3) * 128, :])
                xg[g + 2] = t
            xts = xg[g]
            lhsTs = [xts[:, kt * gc + gloc * 128:kt * gc + (gloc + 1) * 128]
                     for kt in range(KT)]
            ph0 = pph0.tile([128, 512], f32, tag="py0")
            ph1 = pph1.tile([128, 512], f32, tag="py1")
            # kt0 of h0 is emitted in part1 (before the previous tile's
            # injections) so its stationary load hides under a long stream:
            # the inject pair's second matmul streams only a few ns, which
            # otherwise exposes ~106ns of LDWEIGHTS on the next matmul
            nc.tensor.matmul(ph0[:], lhsTs[0], wslice(0, 0),
                             start=True, stop=False)
            cur["part2"] = (mt, xts, gloc, lhsTs, ph0, ph1)

        def emit_main2(mt):
            _mt, xts, gloc, lhsTs, ph0, ph1 = cur["part2"]
            assert _mt == mt
            for kt in range(1, KT):
                nc.tensor.matmul(ph0[:], lhsTs[kt], wslice(kt, 0),
                                 start=False, stop=False)
            if gloc == 0:
                # group stats: one [49, gc] matmul set per group
                pt = pps.tile([SW, gc], f32, tag="pt")
                for kt in range(KT):
                    nc.tensor.matmul(
                        pt[:], wgb_sb[:, kt * SW:(kt + 1) * SW],
                        xts[:, kt * gc:(kt + 1) * gc],
                        start=(kt == 0), stop=(kt == KT - 1))
                cur["pt"] = pt
            # stage this tile's [-mean | 1] rows to SBUF fp16 for the
            # injection: Identity activation adds the constant ones rows via
            # the per-partition bias mask (stats rows of the mask are 0).
            ext = extp.tile([SW, 128], f16, tag="ext")
            nc.scalar.activation(
                out=ext[:], in_=cur["pt"][:, gloc * 128:(gloc + 1) * 128],
                func=mybir.ActivationFunctionType.Identity,
                bias=ones_sb[0:SW, :], scale=1.0)
            for kt in range(KT):
                nc.tensor.matmul(ph1[:], lhsTs[kt], wslice(kt, 1),
                                 start=(kt == 0), stop=False)
            state_a[mt] = (ph0, ph1, ext)

        def emit_epi_a(mt):
            ph0, ph1, ext = state_a.pop(mt)
            # inject (b - mean) into the y PSUM: rank-17 matmul per half
            nc.tensor.matmul(ph0[:], ext[0:17, :], gb_sb[0:17, 0:512],
                             start=False, stop=True)
            nc.tensor.matmul(ph1[:], ext[32:SW, :], gb_sb[32:SW, 512:N],
                             start=False, stop=True)
            # variance: square (Scalar, fp16 out) -> two-level fp16 pair-fold
            # at the DVE 2x packed rate -> short segmented reduce.  Folds
            # live in THIS stage so next round's DVE starts with the applies
            # (whose sqrt input is already done), freeing PSUM banks a full
            # round earlier.
            ysq = epi.tile([128, N], f16, tag="ysq")
            nc.scalar.square(ysq[:, 0:512], ph0[:])
            nc.scalar.square(ysq[:, 512:N], ph1[:])
            ysq3 = ysq[:].rearrange("p (g e) -> p g e", e=GS)
            t2 = epi.tile([128, N // 2], f16, tag="t2")
            nc.vector.tensor_add(
                t2[:].rearrange("p (g e) -> p g e", e=GS // 2),
                ysq3[:, :, 0:GS // 2], ysq3[:, :, GS // 2:GS])
            Q = epi.tile([128, NG], f16, tag="Q")
            with nc.allow_low_precision(reason="fp16 group sums of squares; "
                                        "var rel err ~1e-3 vs 2e-2 budget"):
                nc.vector.tensor_reduce(
                    out=Q[:],
                    in_=t2[:].rearrange("p (g e) -> p g e", e=GS // 2),
                    axis=mybir.AxisListType.X, op=Alu.add)
            state_b[mt] = (ph0, ph1, Q)

        def emit_epi_b(mt):
            ph0, ph1, Q = state_b.pop(mt)
            # group std = sqrt(Q/32 + eps): scale+bias fold into the ACT sqrt
            s = epi.tile([128, NG], f32, tag="s")
            nc.scalar.activation(
                out=s[:], in_=Q[:], func=mybir.ActivationFunctionType.Sqrt,
                bias=eps_sb[:], scale=1.0 / GS)
            # apply per half: out = clip(y'/std, -2, 2), fused recip+clip;
            # h0's psum frees one apply earlier than h1's
            o = outp.tile([128, N], f16, tag="o")
            for h, ph in ((0, ph0), (1, ph1)):
                sh = bass.AP(tensor=s.tensor, offset=s.offset + 16 * h,
                             ap=[s.ap[0], [1, 16], [0, GS]])
                nc.vector._custom_dve(
                    ops["apply"],
                    out=o[:, 512 * h:512 * (h + 1)].rearrange(
                        "p (g e) -> p g e", e=GS),
                    in0=ph[:].rearrange("p (g e) -> p g e", e=GS),
                    in1=sh, s0=-0.23549792, s1=2.0017324)
            if apply_affine:
                nc.vector.tensor_mul(o[:], o[:], gam_sb[:])
                nc.vector.tensor_add(o[:], o[:], bet_sb[:])
                nc.vector.tensor_scalar(
                    out=o[:], in0=o[:], scalar1=-HT, scalar2=HT,
                    op0=Alu.max, op1=Alu.min)
            # the tail's bunched output DMAs alternate onto the scalar queue
            # (idle by then) so the final transfers drain in parallel
            dma_eng = nc.scalar if (mt >= n_tiles - 8 and mt % 2) else nc.sync
            dma_eng.dma_start(out=out_d[mt * 128:(mt + 1) * 128, :], in_=o[:])

        # oldest-tile work first on every engine so short late-stage ops are
        # not queued behind long earlier-stage ops of newer tiles
        for mt in range(n_tiles):
            if mt >= 2:
                emit_epi_b(mt - 2)
            emit_main(mt)
            if mt >= 1:
                emit_epi_a(mt - 1)
            emit_main2(mt)
        if n_tiles >= 2:
            emit_epi_b(n_tiles - 2)
        emit_epi_a(n_tiles - 1)
        emit_epi_b(n_tiles - 1)

    nc.finalize()
    return nc


def _prep_host(x_shard_t16, weight, bias, m_loc):
    bf = np.float16
    n_tiles = m_loc // 128
    gsz = min(GRP, n_tiles)
    gc = gsz * 128
    n_groups = n_tiles // gsz
    # x.T packed per group, kt-major within a row
    xtp_h = np.ascontiguousarray(
        x_shard_t16.reshape(KT, 128, n_groups, gc)
        .transpose(2, 1, 0, 3).reshape(n_groups * 128, KT * gc))
    return xtp_h


def _prep_host_const(weight, bias):
    bf = np.float16
    wtT = np.ascontiguousarray(weight.T.astype(bf))  # [K, N]
    # packed weights [128, 4096]: kt-major h0 halves then h1 halves
    wtp_h = np.zeros((128, 2 * KT * 512), dtype=bf)
    for kt in range(KT):
        wtp_h[:, kt * 512:(kt + 1) * 512] = wtT[kt * 128:(kt + 1) * 128,
                                                0:512]
        wtp_h[:, 2048 + kt * 512:2048 + (kt + 1) * 512] = \
            wtT[kt * 128:(kt + 1) * 128, 512:N]
    # stats stationary: per k-tile columns = -(1/32) * group-sum of weights,
    # already transposed ([K, group]); ones/pad columns stay 0.  Packed
    # kt-major into [128, KT*SW].
    wg = weight.reshape(NG, GS, K).sum(axis=1) * (-1.0 / GS)  # [NG, K]
    wgb_h = np.zeros((128, KT * SW), dtype=bf)
    for kt in range(KT):
        wgb_h[:, kt * SW:kt * SW + 16] = \
            wg[0:16, kt * 128:(kt + 1) * 128].T.astype(bf)
        wgb_h[:, kt * SW + 32:kt * SW + 48] = \
            wg[16:32, kt * 128:(kt + 1) * 128].T.astype(bf)
    # injection moving operand: group indicator rows + b'' rows
    b1 = bias.reshape(NG, GS).mean(axis=1)
    bpp = (bias - np.repeat(b1, GS)).astype(np.float64)
    gb_h = np.zeros((SW, N), dtype=bf)
    for g in range(16):
        gb_h[g, g * GS:(g + 1) * GS] = np.float16(1.0)
        gb_h[32 + g, 512 + g * GS:512 + (g + 1) * GS] = np.float16(1.0)
    gb_h[16, 0:512] = bpp[0:512].astype(bf)
    gb_h[48, 512:1024] = bpp[512:1024].astype(bf)
    msk_h = np.zeros((128, 1), dtype=np.float32)
    msk_h[16, 0] = 1.0
    msk_h[48, 0] = 1.0
    return wtp_h, wgb_h, gb_h, msk_h


def run(x, weight, bias, gamma, beta, m_loc=None, trace=False):
    from concourse.bass_utils import run_bass_kernel_spmd

    bf = np.float16
    x = np.asarray(x, dtype=np.float32)
    weight = np.asarray(weight, dtype=np.float32)
    bias = np.asarray(bias, dtype=np.float32)
    gamma = np.asarray(gamma, dtype=np.float32)
    beta = np.asarray(beta, dtype=np.float32)

    m_total = x.shape[0]
    if m_loc is None:
        m_loc = m_total // N_CORES
    assert m_total == m_loc * N_CORES

    apply_affine = not (np.all(gamma == 1.0) and np.all(beta == 0.0))
    nc = build(m_loc, apply_affine)
    wtp_h, wgb_h, gb_h, msk_h = _prep_host_const(weight, bias)

    xt16 = x.T.astype(bf)  # [K, m_total]
    in_maps = []
    for c in range(N_CORES):
        m = {
            "xtp": _prep_host(
                np.ascontiguousarray(xt16[:, c * m_loc:(c + 1) * m_loc]),
                weight, bias, m_loc),
            "wtp": wtp_h, "wgb": wgb_h, "gb": gb_h, "msk": msk_h,
        }
        if apply_affine:
            m["gam"] = np.ascontiguousarray(np.broadcast_to(gamma, (128, N)))
            m["bet"] = np.ascontiguousarray(np.broadcast_to(beta, (128, N)))
        in_maps.append(m)

    res = run_bass_kernel_spmd(nc, in_maps, list(range(N_CORES)), trace=trace)
    out = np.concatenate([res.results[c]["out"] for c in range(N_CORES)],
                         axis=0).astype(np.float32)
    return out, res


def kernel(x, weight, bias, gamma, beta):
    out, _ = run(x, weight, bias, gamma, beta)
    return out


# revision 51
# speedup vs baseline: 1.0340x; 1.0340x over previous
"""Fused Linear + GroupNorm + Hardtanh kernel for Trainium2 (8 NeuronCores).

Problem: out = clip(groupnorm(x @ W.T + b, 32 groups), -2, 2), with
x [65536, 512] fp32, W [1024, 512] fp32, gamma=1/beta=0.

Strategy (data-parallel over the 8 cores, 8192 rows each):
 - Host pre-transposes x, casts matmul operands to fp16 (PSUM accum
   stays fp32), and packs x.T into per-group tiles: one [128, 4*512]
   SBUF tile per 4 m-tiles, kt-major, so each group is ONE 128-descriptor
   DMA (~650ns queue dispatch) and every matmul/stats slice is a simple
   contiguous slice of it.  W.T is packed the same way ([128, 4096],
   h0 halves then h1 halves) and stays SBUF-resident.
 - Group sums of y come from a second, *transposed* stats matmul per
   4-tile group (stationary = per-k-tile group-summed weights scaled by
   -1/32, moving = the group's 512 x columns), so the [-mean | 1]
   stationary needed by the mean/bias injection comes out of PSUM
   already in [group, m] layout.  The constant ones rows are added by
   the PSUM->SBUF staging copy (activation Identity with a
   per-partition bias mask).  Bias enters via the injection's ones row
   as b'' = b - groupmean(b).
 - The injection (rank-17 matmul per N-half) lands (b - mean) into the
   y PSUM, so the epilogue is: square per half (Scalar, fp16 out) ->
   one segmented fp16 tensor_reduce (Vector) -> sqrt (Scalar) -> one
   fused one-Newton-reciprocal + scale + hardtanh-clip custom DVE op
   per half writing fp16 -> DMA (sync queue).
 - y PSUM is split into independent h0/h1 pools (3 x 1 bank each,
   + 2 stats banks = 8) with [h0 x4][stats][h1 x4] matmul emission.
   Per-engine emission runs oldest-pipeline-stage first.
 - Startup: a few zero warmup matmuls start the PE p-state ramp while
   the first DMAs land; queue order puts the first group + kt0 weights
   ahead of everything else.
 - Output is written fp16 and widened to fp32 on the host.
"""
import sys

sys.path.insert(0, "/opt/trn_rl_repo")

import numpy as np

M_FULL, K, N = 65536, 512, 1024
NG, GS = 32, 32
EPS = 1e-5
HT = 2.0
N_CORES = 8
KT = K // 128  # 4 k-tiles
GRP = 4  # m-tiles per group (one x DMA + one stats matmul per group)
SW = 49  # stats width: [16 groups | ones | pad...] @0, [16 groups | ones] @32
N_WARM = 6  # zero matmuls that start the PE p-state ramp during startup

_custom_ops = {}


def _register_custom_ops():
    """Add the fused scale+clip DVE op to the custom-op table (idempotent)."""
    if _custom_ops:
        return _custom_ops
    import concourse.dve_ops as dve_ops
    from concourse.dve_spec import Spec, Src0, Src1, C0, C1, C2, Zero, minn, \
        maxx, lower, _has_src1
    from concourse.dve_uop import DveOpSpec

    def register(name, spec):
        if name in dve_ops._SUB_OPCODE_FOR_NAME:
            return next(o for o in dve_ops.OPS if o.name == name)
        row = max(dve_ops._SUB_OPCODE_FOR_NAME.values()) + 1
        assert row < 0x20
        op = dve_ops.DveOp(name, spec, subdim=False, uops_sha={})
        dve_ops.OPS.append(op)
        dve_ops._SUB_OPCODE_FOR_NAME[name] = row
        dve_ops.CUSTOM_DVE_SPECS[name] = spec
        for ver in ("v3", "v4"):
            uops = lower(spec, ver=ver)
            op.uops_sha[ver] = DveOpSpec(
                name=name, opcode=row, uops=uops,
                rd1_en=_has_src1(spec)).sha(ver)
        return op

    # out = clip(in0 / in1, -imm2, imm2): one-Newton fast reciprocal of the
    # broadcast group-std (Src1) fused with the scale and the hardtanh clip.
    # 8/8 ALU stages; reciprocal rel err ~1.7e-3.
    from concourse.dve_spec import Bin, AluOp
    y0 = Bin(AluOp.BITWISE_NOT, Src1, Src1) * C0
    y1 = y0 * (C1 - Src1 * y0)
    # the clip reuses the Newton constant C1=2.0017324 as the bound (8-stage
    # budget): clipping at +-2.0017 instead of +-2.0 adds <=1.7e-3 abs error

    def _ref_apply(in0, in1, s0, s1, imm2):
        x = np.ascontiguousarray(in1.astype(np.float32))
        nx = (~x.view(np.int32)).view(np.float32)
        y0r = nx * s0
        y1r = y0r * (s1 - x * y0r)
        return np.minimum(np.maximum(in0.astype(np.float32) * y1r, -s1), s1)

    _custom_ops["apply"] = register("APPLY_RECIP_CLIP_ANT", Spec(
        body=minn(maxx(Src0 * y1, Zero - C1), C1),
        reference=_ref_apply))
    return _custom_ops


def build(m_loc: int, apply_affine: bool):
    import concourse.bass as bass
    import concourse.mybir as mybir
    import concourse.tile as tile
    from concourse import bacc
    from contextlib import ExitStack

    ops = _register_custom_ops()
    f32 = mybir.dt.float32
    f16 = mybir.dt.float16
    Alu = mybir.AluOpType
    n_tiles = m_loc // 128
    gsz = min(GRP, n_tiles)  # m-tiles per group
    gc = gsz * 128  # x columns per group
    n_groups = n_tiles // gsz

    nc = bacc.Bacc()
    # x.T packed per group: row g*128+p, col kt*gc+c = x[g*gc+c, kt*128+p]
    xtp_d = nc.dram_tensor("xtp", [n_groups * 128, KT * gc], f16,
                           kind="ExternalInput")
    # packed weights: [128, 4096] = kt-major h0 halves then h1 halves
    wtp_d = nc.dram_tensor("wtp", [128, 2 * KT * 512], f16,
                           kind="ExternalInput")
    wgb_d = nc.dram_tensor("wgb", [128, KT * SW], f16, kind="ExternalInput")
    gb_d = nc.dram_tensor("gb", [SW, N], f16, kind="ExternalInput")
    msk_d = nc.dram_tensor("msk", [128, 1], f32, kind="ExternalInput")
    if apply_affine:
        gam_d = nc.dram_tensor("gam", [128, N], f32, kind="ExternalInput")
        bet_d = nc.dram_tensor("bet", [128, N], f32, kind="ExternalInput")
    out_d = nc.dram_tensor("out", [m_loc, N], f16, kind="ExternalOutput")

    with tile.TileContext(nc) as tc, ExitStack() as ctx:
        const = ctx.enter_context(tc.tile_pool(name="const", bufs=1))
        xpool = ctx.enter_context(tc.tile_pool(name="xts", bufs=4))
        # pool declaration order sets PSUM bank placement: stats at bank 0,
        # h0 in banks 1-3, h1 in banks 4-7.  Measured: this keeps BOTH the
        # stats matmuls (216ns) and the injection pairs (~320ns) at full
        # rate; pph0=4 or stats-last configs slow one of them by ~50%.
        pps = ctx.enter_context(tc.tile_pool(name="pps", bufs=1, space="PSUM"))
        pph0 = ctx.enter_context(tc.tile_pool(name="pph0", bufs=3,
                                              space="PSUM"))
        pph1 = ctx.enter_context(tc.tile_pool(name="pph1", bufs=4,
                                              space="PSUM"))
        epi = ctx.enter_context(tc.tile_pool(name="epi", bufs=3))
        extp = ctx.enter_context(tc.tile_pool(name="extp", bufs=3))
        outp = ctx.enter_context(tc.tile_pool(name="outp", bufs=4))

        # --- PE p-state warmup: zero matmuls keep the Tensor engine busy
        # (ramping its clock) while the first weight/x DMAs land ---
        # gpsimd memset issues ~1.3us earlier than vector at boot, so the
        # PE clock ramp starts sooner
        warm_sb = const.tile([128, 512], f16, tag="warm")
        nc.gpsimd.memset(warm_sb[:], 0.0)
        warm_ps = pps.tile([SW, 512], f32, tag="pt")
        for _ in range(N_WARM):
            nc.tensor.matmul(warm_ps[0:1, :], warm_sb[:, 0:1], warm_sb[:],
                             start=True, stop=True)

        # --- resident constants + first x groups.  Queue order = startup
        # priority: scalar carries group 0 then the small stats/inject
        # constants; sync carries the weights then group 1. ---
        xg = [None] * n_groups
        # group 0 arrives in kt-sized pieces (disjoint column ranges) so the
        # first main matmuls gate on as little data as possible
        xg[0] = xpool.tile([128, KT * gc], f16, tag="xts", name="xg0")
        nc.scalar.dma_start(out=xg[0][:, 0:gc], in_=xtp_d[0:128, 0:gc])
        nc.scalar.dma_start(out=xg[0][:, gc:2 * gc],
                            in_=xtp_d[0:128, gc:2 * gc])
        nc.scalar.dma_start(out=xg[0][:, 2 * gc:4 * gc],
                            in_=xtp_d[0:128, 2 * gc:4 * gc])
        wgb_sb = const.tile([128, KT * SW], f16, tag="wgb")
        nc.scalar.dma_start(out=wgb_sb[:], in_=wgb_d[:])
        # per-partition bias mask for the staging copy: 1.0 at the ones rows
        ones_sb = const.tile([128, 1], f32, tag="onesmask")
        nc.scalar.dma_start(out=ones_sb[:], in_=msk_d[:])
        gb_sb = const.tile([SW, N], f16, tag="gb")
        nc.scalar.dma_start(out=gb_sb[:], in_=gb_d[:])

        wt_sb = const.tile([128, 2 * KT * 512], f16, tag="wt")
        nc.sync.dma_start(out=wt_sb[:, 0:512], in_=wtp_d[:, 0:512])
        nc.sync.dma_start(out=wt_sb[:, 512:2048], in_=wtp_d[:, 512:2048])
        nc.sync.dma_start(out=wt_sb[:, 2048:2560], in_=wtp_d[:, 2048:2560])
        nc.sync.dma_start(out=wt_sb[:, 2560:4096], in_=wtp_d[:, 2560:4096])
        if n_groups > 1:
            xg[1] = xpool.tile([128, KT * gc], f16, tag="xts", name="xg1")
            nc.sync.dma_start(out=xg[1][:], in_=xtp_d[128:256, :])

        eps_sb = const.tile([128, 1], f32, tag="eps")
        nc.vector.memset(eps_sb[:], EPS)
        if apply_affine:
            gam_sb = const.tile([128, N], f32, tag="gam")
            nc.scalar.dma_start(out=gam_sb[:], in_=gam_d[:])
            bet_sb = const.tile([128, N], f32, tag="bet")
            nc.scalar.dma_start(out=bet_sb[:], in_=bet_d[:])

        state_a = {}
        state_b = {}
        cur = {"pt": None}

        def wslice(kt, h):
            return wt_sb[:, (h * KT + kt) * 512:(h * KT + kt + 1) * 512]

        def emit_main(mt):
            g, gloc = divmod(mt, gsz)
            if gloc == 0 and g + 2 < n_groups and xg[g + 2] is None:
                # prefetch group g+2 (4-deep pool: target buffer long freed)
                t = xpool.tile([128, KT * gc], f16, tag="xts", name="xgp")
                nc.sync.dma_start(
                    out=t[:], in_=xtp_d[(g + 2) * 128:(g + 3) * 128, :])
                xg[g + 2] = t
            xts = xg[g]
            lhsTs = [xts[:, kt * gc + gloc * 128:kt * gc + (gloc + 1) * 128]
                     for kt in range(KT)]
            ph0 = pph0.tile([128, 512], f32, tag="py0")
            ph1 = pph1.tile([128, 512], f32, tag="py1")
            for kt in range(KT):
                nc.tensor.matmul(ph0[:], lhsTs[kt], wslice(kt, 0),
                                 start=(kt == 0), stop=False)
            if gloc == 0:
                # group stats: one [49, gc] matmul set per group
                pt = pps.tile([SW, gc], f32, tag="pt")
                for kt in range(KT):
                    nc.tensor.matmul(
                        pt[:], wgb_sb[:, kt * SW:(kt + 1) * SW],
                        xts[:, kt * gc:(kt + 1) * gc],
                        start=(kt == 0), stop=(kt == KT - 1))
                cur["pt"] = pt
            # stage this tile's [-mean | 1] rows to SBUF fp16 for the
            # injection: Identity activation adds the constant ones rows via
            # the per-partition bias mask (stats rows of the mask are 0).
            ext = extp.tile([SW, 128], f16, tag="ext")
            nc.scalar.activation(
                out=ext[:], in_=cur["pt"][:, gloc * 128:(gloc + 1) * 128],
                func=mybir.ActivationFunctionType.Identity,
                bias=ones_sb[0:SW, :], scale=1.0)
            for kt in range(KT):
                nc.tensor.matmul(ph1[:], lhsTs[kt], wslice(kt, 1),
                                 start=(kt == 0), stop=False)
            state_a[mt] = (ph0, ph1, ext)

        def emit_epi_a(mt):
            ph0, ph1, ext = state_a.pop(mt)
            # inject (b - mean) into the y PSUM: rank-17 matmul per half
            nc.tensor.matmul(ph0[:], ext[0:17, :], gb_sb[0:17, 0:512],
                             start=False, stop=True)
            nc.tensor.matmul(ph1[:], ext[32:SW, :], gb_sb[32:SW, 512:N],
                             start=False, stop=True)
            # variance: square (Scalar, fp16 out) -> two-level fp16 pair-fold
            # at the DVE 2x packed rate -> short segmented reduce.  Folds
            # live in THIS stage so next round's DVE starts with the applies
            # (whose sqrt input is already done), freeing PSUM banks a full
            # round earlier.
            ysq = epi.tile([128, N], f16, tag="ysq")
            nc.scalar.square(ysq[:, 0:512], ph0[:])
            nc.scalar.square(ysq[:, 512:N], ph1[:])
            ysq3 = ysq[:].rearrange("p (g e) -> p g e", e=GS)
            t2 = epi.tile([128, N // 2], f16, tag="t2")
            nc.vector.tensor_add(
                t2[:].rearrange("p (g e) -> p g e", e=GS // 2),
                ysq3[:, :, 0:GS // 2], ysq3[:, :, GS // 2:GS])
            t23 = t2[:].rearrange("p (g e) -> p g e", e=GS // 2)
            t4 = epi.tile([128, N // 4], f16, tag="t4")
            nc.vector.tensor_add(
                t4[:].rearrange("p (g e) -> p g e", e=GS // 4),
                t23[:, :, 0:GS // 4], t23[:, :, GS // 4:GS // 2])
            Q = epi.tile([128, NG], f16, tag="Q")
            with nc.allow_low_precision(reason="fp16 group sums of squares; "
                                        "var rel err ~1e-3 vs 2e-2 budget"):
                nc.vector.tensor_reduce(
                    out=Q[:],
                    in_=t4[:].rearrange("p (g e) -> p g e", e=GS // 4),
                    axis=mybir.AxisListType.X, op=Alu.add)
            state_b[mt] = (ph0, ph1, Q)

        def emit_epi_b(mt):
            ph0, ph1, Q = state_b.pop(mt)
            # group std = sqrt(Q/32 + eps): scale+bias fold into the ACT sqrt
            s = epi.tile([128, NG], f32, tag="s")
            nc.scalar.activation(
                out=s[:], in_=Q[:], func=mybir.ActivationFunctionType.Sqrt,
                bias=eps_sb[:], scale=1.0 / GS)
            # apply per half: out = clip(y'/std, -2, 2), fused recip+clip;
            # h0's psum frees one apply earlier than h1's
            o = outp.tile([128, N], f16, tag="o")
            for h, ph in ((0, ph0), (1, ph1)):
                sh = bass.AP(tensor=s.tensor, offset=s.offset + 16 * h,
                             ap=[s.ap[0], [1, 16], [0, GS]])
                nc.vector._custom_dve(
                    ops["apply"],
                    out=o[:, 512 * h:512 * (h + 1)].rearrange(
                        "p (g e) -> p g e", e=GS),
                    in0=ph[:].rearrange("p (g e) -> p g e", e=GS),
                    in1=sh, s0=-0.23549792, s1=2.0017324)
            if apply_affine:
                nc.vector.tensor_mul(o[:], o[:], gam_sb[:])
                nc.vector.tensor_add(o[:], o[:], bet_sb[:])
                nc.vector.tensor_scalar(
                    out=o[:], in0=o[:], scalar1=-HT, scalar2=HT,
                    op0=Alu.max, op1=Alu.min)
            # the tail's bunched output DMAs alternate onto the scalar queue
            # (idle by then) so the final transfers drain in parallel
            dma_eng = nc.scalar if (mt >= n_tiles - 8 and mt % 2) else nc.sync
            dma_eng.dma_start(out=out_d[mt * 128:(mt + 1) * 128, :], in_=o[:])

        # oldest-tile work first on every engine so short late-stage ops are
        # not queued behind long earlier-stage ops of newer tiles
        for mt in range(n_tiles):
            if mt >= 2:
                emit_epi_b(mt - 2)
            if mt >= 1:
                emit_epi_a(mt - 1)
            emit_main(mt)
        if n_tiles >= 2:
            emit_epi_b(n_tiles - 2)
        emit_epi_a(n_tiles - 1)
        emit_epi_b(n_tiles - 1)

    nc.finalize()
    return nc


def _prep_host(x_shard_t16, weight, bias, m_loc):
    bf = np.float16
    n_tiles = m_loc // 128
    gsz = min(GRP, n_tiles)
    gc = gsz * 128
    n_groups = n_tiles // gsz
    # x.T packed per group, kt-major within a row
    xtp_h = np.ascontiguousarray(
        x_shard_t16.reshape(KT, 128, n_groups, gc)
        .transpose(2, 1, 0, 3).reshape(n_groups * 128, KT * gc))
    return xtp_h


def _prep_host_const(weight, bias):
    bf = np.float16
    wtT = np.ascontiguousarray(weight.T.astype(bf))  # [K, N]
    # packed weights [128, 4096]: kt-major h0 halves then h1 halves
    wtp_h = np.zeros((128, 2 * KT * 512), dtype=bf)
    for kt in range(KT):
        wtp_h[:, kt * 512:(kt + 1) * 512] = wtT[kt * 128:(kt + 1) * 128,
                                                0:512]
        wtp_h[:, 2048 + kt * 512:2048 + (kt + 1) * 512] = \
            wtT[kt * 128:(kt + 1) * 128, 512:N]
    # stats stationary: per k-tile columns = -(1/32) * group-sum of weights,
    # already transposed ([K, group]); ones/pad columns stay 0.  Packed
    # kt-major into [128, KT*SW].
    wg = weight.reshape(NG, GS, K).sum(axis=1) * (-1.0 / GS)  # [NG, K]
    wgb_h = np.zeros((128, KT * SW), dtype=bf)
    for kt in range(KT):
        wgb_h[:, kt * SW:kt * SW + 16] = \
            wg[0:16, kt * 128:(kt + 1) * 128].T.astype(bf)
        wgb_h[:, kt * SW + 32:kt * SW + 48] = \
            wg[16:32, kt * 128:(kt + 1) * 128].T.astype(bf)
    # injection moving operand: group indicator rows + b'' rows
    b1 = bias.reshape(NG, GS).mean(axis=1)
    bpp = (bias - np.repeat(b1, GS)).astype(np.float64)
    gb_h = np.zeros((SW, N), dtype=bf)
    for g in range(16):
        gb_h[g, g * GS:(g + 1) * GS] = np.float16(1.0)
        gb_h[32 + g, 512 + g * GS:512 + (g + 1) * GS] = np.float16(1.0)
    gb_h[16, 0:512] = bpp[0:512].astype(bf)
    gb_h[48, 512:1024] = bpp[512:1024].astype(bf)
    msk_h = np.zeros((128, 1), dtype=np.float32)
    msk_h[16, 0] = 1.0
    msk_h[48, 0] = 1.0
    return wtp_h, wgb_h, gb_h, msk_h


def run(x, weight, bias, gamma, beta, m_loc=None, trace=False):
    from concourse.bass_utils import run_bass_kernel_spmd

    bf = np.float16
    x = np.asarray(x, dtype=np.float32)
    weight = np.asarray(weight, dtype=np.float32)
    bias = np.asarray(bias, dtype=np.float32)
    gamma = np.asarray(gamma, dtype=np.float32)
    beta = np.asarray(beta, dtype=np.float32)

    m_total = x.shape[0]
    if m_loc is None:
        m_loc = m_total // N_CORES
    assert m_total == m_loc * N_CORES

    apply_affine = not (np.all(gamma == 1.0) and np.all(beta == 0.0))
    nc = build(m_loc, apply_affine)
    wtp_h, wgb_h, gb_h, msk_h = _prep_host_const(weight, bias)

    xt16 = x.T.astype(bf)  # [K, m_total]
    in_maps = []
    for c in range(N_CORES):
        m = {
            "xtp": _prep_host(
                np.ascontiguousarray(xt16[:, c * m_loc:(c + 1) * m_loc]),
                weight, bias, m_loc),
            "wtp": wtp_h, "wgb": wgb_h, "gb": gb_h, "msk": msk_h,
        }
        if apply_affine:
            m["gam"] = np.ascontiguousarray(np.broadcast_to(gamma, (128, N)))
            m["bet"] = np.ascontiguousarray(np.broadcast_to(beta, (128, N)))
        in_maps.append(m)

    res = run_bass_kernel_spmd(nc, in_maps, list(range(N_CORES)), trace=trace)
    out = np.concatenate([res.results[c]["out"] for c in range(N_CORES)],
                         axis=0).astype(np.float32)
    return out, res


def kernel(x, weight, bias, gamma, beta):
    out, _ = run(x, weight, bias, gamma, beta)
    return out
